# revision 11
# baseline (speedup 1.0000x reference)
"""MIL gated-attention pooling kernel for Trainium2 (8 NeuronCores, SPMD).

Problem (per reference):
    A_pre = tanh(x@W1 + b1) * sigmoid(x@W3 + b3)      # [N, H]
    A     = A_pre @ W2 + b2                           # [N, K]
    P     = softmax over instances per (bag, head)    # [B, K, L]
    out   = einsum('bkl,bld->bkd', P, x) -> [B, K*D]

Shapes hardcoded: B=32 bags, L=2048 instances/bag, D=512, H=256, K=4.
Sharding: data-parallel over bags, 4 bags (8192 rows) per core, weights
replicated. No cross-core communication.

Device algorithm per core (v3):
  - 8 supersteps of 1024 instances; gated-MLP hidden tiles live in 2-bank
    PSUM tiles [128, 2, 512] so each tanh covers 1024 columns in ONE
    activation instruction (per-instruction ACT overhead is ~185ns).
  - sigmoid(h) = 0.5*(1+tanh(h/2)); 0.5 folded into W2 host-side; the
    gate (1+s)*t is ONE DVE scalar_tensor_tensor op.
  - b2 and softmax max-subtraction dropped (both cancel in softmax).
  - all DMA triggers issue from the idle Pool engine (~25ns each vs
    ~600ns on SP, whose serial trigger stream delayed the first matmul
    by ~8us in v2).
  - exp() runs per superstep so the weighted sum trails the GEMM by two
    supersteps and the epilogue is short.
  - A@W2 and weighted-sum matmuls are interleaved between the GEMM's
    DoubleRow matmuls to keep the PE stream dense (p-state: idle gaps
    drop the PE clock 2.4 -> 1.2GHz).
  - MODE "bf16": x for the weighted sum streams as bf16 [12.6MB/core].
    MODE "fp8": it streams as fp8 [8.4MB/core] and exp-weights are split
    e = e1 + e2 (both fp8, zero-padded to 16 lanes for walrus' DoubleRow
    codegen) so the weighted sum loses only x's fp8 quantization.
"""

import numpy as np
import ml_dtypes
from contextlib import ExitStack

B, L, D, H, K = 32, 2048, 512, 256, 4
NCORES = 8
BPC = B // NCORES       # bags per core = 4
R = BPC * L             # rows per core = 8192
SS = 8                  # supersteps per core (1024 instances each)
NI = 1024               # instances per superstep
DC = D // 128           # contraction chunks = 4
NCH = L // 128          # 128-row chunks per bag = 16

_BF16 = ml_dtypes.bfloat16
_FP8 = ml_dtypes.float8_e4m3

MODE = "fp8"            # "bf16" (safe) | "fp8" (fast; err ~1.74e-2 vs 2e-2 gate)
_CACHE = {}


def _build_nc(mode):
    import concourse.bacc as bacc
    import concourse.tile as tile
    import concourse.mybir as mybir
    import concourse.bass as bass

    dt = mybir.dt
    AF = mybir.ActivationFunctionType
    DR = mybir.MatmulPerfMode.DoubleRow
    fp8 = mode == "fp8"

    nc = bacc.Bacc("TRN2", target_bir_lowering=False, debug=False)

    # xt[ss, p, 2*dc2+r, n] = fp8(x[ss*1024+n, dc2*256+r*128+p])
    xt_d = nc.dram_tensor("xt", [SS, 128, DC, NI], dt.float8e4, kind="ExternalInput").ap()
    if fp8:
        # q1[ss, p, pr, j, d] = fp8(x[ss*1024+(2*pr+j)*128+p, d])
        xa_d = nc.dram_tensor("xa", [SS, 128, 4, 2, D], dt.float8e4, kind="ExternalInput").ap()
        ones_d = nc.dram_tensor("ones", [128, 2, 1], dt.float8e4, kind="ExternalInput").ap()
    else:
        # xa[ss, p, c, d] = bf16(x[ss*1024+c*128+p, d])
        xa_d = nc.dram_tensor("xa", [SS, 128, 8, D], dt.bfloat16, kind="ExternalInput").ap()
        ones_d = nc.dram_tensor("ones", [128, 1], dt.bfloat16, kind="ExternalInput").ap()
    # w13[p, dc2, r, h'] = 16*[W1|W3][dc2*256+r*128+p, h']
    w13_d = nc.dram_tensor("w13", [128, 2, 2, 2 * H], dt.float8e4, kind="ExternalInput").ap()
    w2_d = nc.dram_tensor("w2", [128, 2, K], dt.bfloat16, kind="ExternalInput").ap()
    b13_d = nc.dram_tensor("b13", [128, DC], dt.float32, kind="ExternalInput").ap()
    out_d = nc.dram_tensor("out", [BPC, K, D], dt.float32, kind="ExternalOutput").ap()

    with tile.TileContext(nc) as tc, ExitStack() as ctx:
        consts = ctx.enter_context(tc.tile_pool(name="consts", bufs=1))
        xtp = ctx.enter_context(tc.tile_pool(name="xtp", bufs=3))
        xap = ctx.enter_context(tc.tile_pool(name="xap", bufs=6))
        tsp = ctx.enter_context(tc.tile_pool(name="tsp", bufs=4))
        app = ctx.enter_context(tc.tile_pool(name="app", bufs=4))
        epool = ctx.enter_context(tc.tile_pool(name="epool", bufs=3))
        opool = ctx.enter_context(tc.tile_pool(name="opool", bufs=2))
        rpool = ctx.enter_context(tc.tile_pool(name="rpool", bufs=2))

        psH = ctx.enter_context(tc.tile_pool(name="psH", bufs=2, space=bass.MemorySpace.PSUM))
        psA = ctx.enter_context(tc.tile_pool(name="psA", bufs=2, space=bass.MemorySpace.PSUM))
        psU = ctx.enter_context(tc.tile_pool(name="psU", bufs=1, space=bass.MemorySpace.PSUM))
        psZ = ctx.enter_context(tc.tile_pool(name="psZ", bufs=1, space=bass.MemorySpace.PSUM))

        dma = nc.gpsimd.dma_start  # Pool-engine triggers: ~25ns vs ~600ns on SP

        # constants + first superstep, ordered so the first GEMM matmul
        # (needs w13[dc2=0] + xt0 first half) unblocks as early as possible
        w13_sb = consts.tile([128, 2, 2, 2 * H], dt.float8e4)
        dma(out=w13_sb[:, 0], in_=w13_d[:, 0])

        xt_tiles = {}
        xa_tiles = {}

        def fetch(ss, split=False):
            if ss >= SS or ss in xt_tiles:
                return
            xtt = xtp.tile([128, DC, NI], dt.float8e4, tag="xt", name="xtt")
            if split:
                dma(out=xtt[:, 0:2, :], in_=xt_d[ss, :, 0:2, :])
                dma(out=xtt[:, 2:4, :], in_=xt_d[ss, :, 2:4, :])
            else:
                dma(out=xtt[:], in_=xt_d[ss])
            xt_tiles[ss] = xtt
            if fp8:
                xat = xap.tile([128, 4, 2, D], dt.float8e4, tag="xa", name="xat")
            else:
                xat = xap.tile([128, 8, D], dt.bfloat16, tag="xa", name="xat")
            dma(out=xat[:], in_=xa_d[ss])
            xa_tiles[ss] = xat

        fetch(0, split=True)
        b13_sb = consts.tile([128, DC], dt.float32)
        dma(out=b13_sb[:], in_=b13_d[:])
        dma(out=w13_sb[:, 1], in_=w13_d[:, 1])
        w2_sb = consts.tile([128, 2, K], dt.bfloat16)
        dma(out=w2_sb[:], in_=w2_d[:])
        if fp8:
            ones_sb = consts.tile([128, 2, 1], dt.float8e4)
        else:
            ones_sb = consts.tile([128, 1], dt.bfloat16)
        dma(out=ones_sb[:], in_=ones_d[:])
        fetch(1)

        # state carried across supersteps
        apts = {}       # ss -> (apt0, apt1)  [128, 2, 512] bf16 (h-chunk 0, 1)
        a_ps_of = {}    # ss -> psum [128, 8*K] logits
        e_of = {}       # ss -> e tile (bf16) or (e1, e2) fp8-padded
        uz_of = {}      # bag -> (u_ps, z_ps)

        def a_w2_quarter(ss, k):
            """A@W2 for chunks (2k, 2k+1) of superstep ss."""
            if ss not in a_ps_of:
                a_ps_of[ss] = psA.tile([128, 8 * K], dt.float32, tag="a", name="a_ps")
            a_ps = a_ps_of[ss]
            apt0, apt1 = apts[ss]
            for c in (2 * k, 2 * k + 1):
                half, cc = c // 4, c % 4
                nc.tensor.matmul(
                    a_ps[:, K * c:K * (c + 1)],
                    apt0[:, half, cc * 128:(cc + 1) * 128],
                    w2_sb[:, 0, :], start=True, stop=False,
                )
                nc.tensor.matmul(
                    a_ps[:, K * c:K * (c + 1)],
                    apt1[:, half, cc * 128:(cc + 1) * 128],
                    w2_sb[:, 1, :], start=False, stop=True,
                )

        def do_exp(ss):
            """exp of superstep ss's logits; alloc u/z at bag start."""
            a_ps = a_ps_of.pop(ss)
            del apts[ss]
            e_b = epool.tile([128, 8, K], dt.bfloat16, tag="e", name="e_b")
            nc.scalar.activation(e_b[:], a_ps[:], AF.Exp)
            if fp8:
                # walrus rejects DoubleRow lhsT narrower than 16 cols, so e1/e2
                # are zero-padded [*, 8, 16]; rows 4..15 of u/z stay zero.
                e1 = epool.tile([128, 8, 16], dt.float8e4, tag="e1", name="e1")
                nc.gpsimd.memset(e1[:, :, K:], 0.0)
                nc.vector.tensor_scalar_add(out=e1[:, :, 0:K], in0=e_b[:], scalar1=0.0)
                e2 = epool.tile([128, 8, 16], dt.float8e4, tag="e2", name="e2")
                nc.gpsimd.memset(e2[:, :, K:], 0.0)
                nc.vector.scalar_tensor_tensor(
                    out=e2[:, :, 0:K], in0=e1[:, :, 0:K], scalar=-1.0, in1=e_b[:],
                    op0=mybir.AluOpType.mult, op1=mybir.AluOpType.add,
                )
                e_of[ss] = (e1, e2)
            else:
                e_of[ss] = e_b
            if ss % 2 == 0:
                if fp8:
                    u_ps = psU.tile([16, D], dt.float32, tag="u", name="u_ps")
                    z_ps = psZ.tile([16, 1], dt.float32, tag="z", name="z_ps")
                else:
                    u_ps = psU.tile([K, D], dt.float32, tag="u", name="u_ps")
                    z_ps = psZ.tile([K, 1], dt.float32, tag="z", name="z_ps")
                uz_of[ss // 2] = (u_ps, z_ps)

        def wsum_quarter(ss, k):
            """Quarter k of superstep ss's exp-weighted sum + normalizer.
            The u/z accumulation group spans the bag (both supersteps)."""
            u_ps, z_ps = uz_of[ss // 2]
            xat = xa_tiles[ss]
            first = ss % 2 == 0 and k == 0
            last = ss % 2 == 1 and k == 3
            if fp8:
                e1, e2 = e_of[ss]
                pr = k  # chunk-pairs 0..3 within the superstep
                nc.tensor.matmul(u_ps[:], e1[:, 2 * pr:2 * pr + 2, :], xat[:, pr],
                                 start=first, stop=False, perf_mode=DR)
                nc.tensor.matmul(u_ps[:], e2[:, 2 * pr:2 * pr + 2, :], xat[:, pr],
                                 start=False, stop=last, perf_mode=DR)
                nc.tensor.matmul(z_ps[:], e1[:, 2 * pr:2 * pr + 2, :], ones_sb[:],
                                 start=first, stop=False, perf_mode=DR)
                nc.tensor.matmul(z_ps[:], e2[:, 2 * pr:2 * pr + 2, :], ones_sb[:],
                                 start=False, stop=last, perf_mode=DR)
            else:
                e_b = e_of[ss]
                for c in (2 * k, 2 * k + 1):  # chunks 0..7 within the superstep
                    lastc = last and c == 2 * k + 1
                    nc.tensor.matmul(u_ps[:], e_b[:, c, :], xat[:, c],
                                     start=first and c == 0, stop=lastc)
                    nc.tensor.matmul(z_ps[:], e_b[:, c, :], ones_sb[:],
                                     start=first and c == 0, stop=lastc)
                    first = False
            if last:
                del e_of[ss - 1], e_of[ss]
                del xa_tiles[ss - 1], xa_tiles[ss]

        def finish(bag):
            u_ps, z_ps = uz_of.pop(bag)
            r_sb = rpool.tile([K, 1], dt.float32, tag="r", name="r_sb")
            nc.vector.reciprocal(out=r_sb[:], in_=z_ps[0:K, :])
            o_sb = opool.tile([K, D], dt.float32, tag="o", name="o_sb")
            nc.vector.tensor_scalar_mul(out=o_sb[:], in0=u_ps[0:K, :], scalar1=r_sb[:])
            dma(out=out_d[bag], in_=o_sb[:])

        for ss in range(SS):
            fetch(ss + 2)
            xtt = xt_tiles.pop(ss)
            tanh_of = {}
            lastss = ss == SS - 1
            for k, bp in enumerate((0, 2, 1, 3)):
                h2 = psH.tile([128, 2, 512], dt.float32, tag="h", name="h2")
                for half in range(2):
                    for dc2 in range(2):
                        nc.tensor.matmul(
                            h2[:, half, :],
                            w13_sb[:, dc2, :, bp * 128:(bp + 1) * 128],
                            xtt[:, 2 * dc2:2 * dc2 + 2, half * 512:(half + 1) * 512],
                            start=(dc2 == 0), stop=(dc2 == 1),
                            perf_mode=DR,
                        )
                # deferred PE work rides between the GEMM's matmul groups
                if ss >= 1:
                    a_w2_quarter(ss - 1, k)
                if ss >= 2:
                    wsum_quarter(ss - 2, k)
                ts = tsp.tile([128, 2, 512], dt.bfloat16, tag="ts", name="ts")
                nc.scalar.activation(
                    ts[:], h2[:], AF.Tanh, bias=b13_sb[:, bp:bp + 1],
                    scale=(1.0 / 16.0 if bp < 2 else 0.5 / 16.0),
                )
                tanh_of[bp] = ts
                if k == 1:  # t(0) and s(2) ready -> gate h-chunk 0
                    apt0 = app.tile([128, 2, 512], dt.bfloat16, tag="ap", name="apt0")
                    nc.vector.scalar_tensor_tensor(
                        out=apt0[:], in0=tanh_of[2][:], scalar=1.0, in1=tanh_of[0][:],
                        op0=mybir.AluOpType.add, op1=mybir.AluOpType.mult,
                    )
                if k == 3:
                    apt1 = app.tile([128, 2, 512], dt.bfloat16, tag="ap", name="apt1")
                    nc.vector.scalar_tensor_tensor(
                        out=apt1[:], in0=tanh_of[3][:], scalar=1.0, in1=tanh_of[1][:],
                        op0=mybir.AluOpType.add, op1=mybir.AluOpType.mult,
                    )
                    apts[ss] = (apt0, apt1)
                    if lastss:  # drain this superstep's own A@W2 inline
                        for k2 in range(4):
                            a_w2_quarter(ss, k2)
            if ss >= 1:
                do_exp(ss - 1)
            if ss >= 3 and ss % 2 == 1:
                finish(ss // 2 - 1)

        # epilogue: exp + weighted sum of the last two supersteps
        do_exp(SS - 1)
        for ss in (SS - 2, SS - 1):
            for k in range(4):
                wsum_quarter(ss, k)
        finish(BPC - 1)

    nc.compile()
    return nc


def get_nc():
    key = "nc_" + MODE
    if key not in _CACHE:
        _CACHE[key] = _build_nc(MODE)
    return _CACHE[key]


def make_in_maps(x, W1, b1, W3, b3, W2, b2):
    x = np.asarray(x, dtype=np.float32)
    W1 = np.asarray(W1, dtype=np.float32)
    W3 = np.asarray(W3, dtype=np.float32)
    W2 = np.asarray(W2, dtype=np.float32)
    b1 = np.asarray(b1, dtype=np.float32)
    b3 = np.asarray(b3, dtype=np.float32)
    fp8 = MODE == "fp8"

    w13 = np.concatenate([W1, W3], axis=1)          # [512, 512]
    # [p, dc2, r, h'] = 16*w13[dc2*256 + r*128 + p, h']
    w13_t = np.ascontiguousarray(
        (16.0 * w13).reshape(2, 2, 128, 2 * H).transpose(2, 0, 1, 3)
    ).astype(_FP8)
    w2_t = np.ascontiguousarray(
        (0.5 * W2).reshape(2, 128, K).transpose(1, 0, 2)
    ).astype(_BF16)
    b13 = np.concatenate([b1, 0.5 * b3]).reshape(DC, 128).T
    b13 = np.ascontiguousarray(b13, dtype=np.float32)
    if fp8:
        ones = np.ones((128, 2, 1), dtype=_FP8)
    else:
        ones = np.ones((128, 1), dtype=_BF16)

    in_maps = []
    for cid in range(NCORES):
        xc = x[cid * R:(cid + 1) * R]               # [8192, 512] fp32
        # xt[ss, p, 2*dc2+r, n] = x[ss*1024+n, dc2*256+r*128+p]
        xt_np = np.ascontiguousarray(
            xc.T.reshape(2, 2, 128, SS, NI).transpose(3, 2, 0, 1, 4).reshape(SS, 128, DC, NI)
        ).astype(_FP8)
        if fp8:
            xa_np = np.ascontiguousarray(
                xc.reshape(SS, 4, 2, 128, D).transpose(0, 3, 1, 2, 4)
            ).astype(_FP8)
        else:
            xa_np = np.ascontiguousarray(
                xc.reshape(SS, 8, 128, D).transpose(0, 2, 1, 3)
            ).astype(_BF16)
        in_maps.append(
            {"xt": xt_np, "xa": xa_np, "w13": w13_t, "w2": w2_t,
             "b13": b13, "ones": ones}
        )
    return in_maps


def kernel(x, W1, b1, W3, b3, W2, b2, bag_lengths):
    from concourse.bass_utils import run_bass_kernel_spmd

    nc = get_nc()
    in_maps = make_in_maps(x, W1, b1, W3, b3, W2, b2)
    res = run_bass_kernel_spmd(nc, in_maps, list(range(NCORES)))
    out = np.empty((B, K * D), dtype=np.float32)
    for c in range(NCORES):
        out[c * BPC:(c + 1) * BPC] = res.results[c]["out"].reshape(BPC, K * D)
    return out


# revision 17
# speedup vs baseline: 1.0986x; 1.0986x over previous
"""MIL gated-attention pooling kernel for Trainium2 (8 NeuronCores, SPMD).

Problem (per reference):
    A_pre = tanh(x@W1 + b1) * sigmoid(x@W3 + b3)      # [N, H]
    A     = A_pre @ W2 + b2                           # [N, K]
    P     = softmax over instances per (bag, head)    # [B, K, L]
    out   = einsum('bkl,bld->bkd', P, x) -> [B, K*D]

Shapes hardcoded: B=32 bags, L=2048 instances/bag, D=512, H=256, K=4.
Sharding: data-parallel over bags, 4 bags (8192 rows) per core, weights
replicated. No cross-core communication.

Device algorithm per core (v3):
  - 8 supersteps of 1024 instances; gated-MLP hidden tiles live in 2-bank
    PSUM tiles [128, 2, 512] so each tanh covers 1024 columns in ONE
    activation instruction (per-instruction ACT overhead is ~185ns).
  - sigmoid(h) = 0.5*(1+tanh(h/2)); 0.5 folded into W2 host-side; the
    gate (1+s)*t is ONE DVE scalar_tensor_tensor op.
  - b2 and softmax max-subtraction dropped (both cancel in softmax).
  - all DMA triggers issue from the idle Pool engine (~25ns each vs
    ~600ns on SP, whose serial trigger stream delayed the first matmul
    by ~8us in v2).
  - exp() runs per superstep so the weighted sum trails the GEMM by two
    supersteps and the epilogue is short.
  - A@W2 and weighted-sum matmuls are interleaved between the GEMM's
    DoubleRow matmuls to keep the PE stream dense (p-state: idle gaps
    drop the PE clock 2.4 -> 1.2GHz).
  - MODE "bf16": x for the weighted sum streams as bf16 [12.6MB/core].
    MODE "fp8": it streams as fp8 [8.4MB/core] and exp-weights are split
    e = e1 + e2 (both fp8, zero-padded to 16 lanes for walrus' DoubleRow
    codegen) so the weighted sum loses only x's fp8 quantization.
"""

import numpy as np
import ml_dtypes
from contextlib import ExitStack

B, L, D, H, K = 32, 2048, 512, 256, 4
NCORES = 8
BPC = B // NCORES       # bags per core = 4
R = BPC * L             # rows per core = 8192
SS = 8                  # supersteps per core (1024 instances each)
NI = 1024               # instances per superstep
DC = D // 128           # contraction chunks = 4
NCH = L // 128          # 128-row chunks per bag = 16

_BF16 = ml_dtypes.bfloat16
_FP8 = ml_dtypes.float8_e4m3

MODE = "fp8"            # "bf16" (safe) | "fp8" (fast; err ~1.74e-2 vs 2e-2 gate)
_CACHE = {}


def _build_nc(mode):
    import concourse.bacc as bacc
    import concourse.tile as tile
    import concourse.mybir as mybir
    import concourse.bass as bass

    dt = mybir.dt
    AF = mybir.ActivationFunctionType
    DR = mybir.MatmulPerfMode.DoubleRow
    fp8 = mode == "fp8"

    nc = bacc.Bacc("TRN2", target_bir_lowering=False, debug=False)

    # xt[ss, p, 2*dc2+r, n] = fp8(x[ss*1024+n, dc2*256+r*128+p])
    xt_d = nc.dram_tensor("xt", [SS, 128, DC, NI], dt.float8e4, kind="ExternalInput").ap()
    if fp8:
        # q1[ss, p, pr, j, d] = fp8(x[ss*1024+(2*pr+j)*128+p, d])
        xa_d = nc.dram_tensor("xa", [SS, 128, 4, 2, D], dt.float8e4, kind="ExternalInput").ap()
        ones_d = nc.dram_tensor("ones", [128, 2, 1], dt.float8e4, kind="ExternalInput").ap()
    else:
        # xa[ss, p, c, d] = bf16(x[ss*1024+c*128+p, d])
        xa_d = nc.dram_tensor("xa", [SS, 128, 8, D], dt.bfloat16, kind="ExternalInput").ap()
        ones_d = nc.dram_tensor("ones", [128, 1], dt.bfloat16, kind="ExternalInput").ap()
    # w13[p, dc2, r, h'] = 16*[W1|W3][dc2*256+r*128+p, h']
    w13_d = nc.dram_tensor("w13", [128, 2, 2, 2 * H], dt.float8e4, kind="ExternalInput").ap()
    w2_d = nc.dram_tensor("w2", [128, 2, K], dt.bfloat16, kind="ExternalInput").ap()
    b13_d = nc.dram_tensor("b13", [128, DC], dt.float32, kind="ExternalInput").ap()
    out_d = nc.dram_tensor("out", [BPC, K, D], dt.float32, kind="ExternalOutput").ap()

    with tile.TileContext(nc) as tc, ExitStack() as ctx:
        consts = ctx.enter_context(tc.tile_pool(name="consts", bufs=1))
        xtp = ctx.enter_context(tc.tile_pool(name="xtp", bufs=3))
        xap = ctx.enter_context(tc.tile_pool(name="xap", bufs=6))
        tsp = ctx.enter_context(tc.tile_pool(name="tsp", bufs=4))
        app = ctx.enter_context(tc.tile_pool(name="app", bufs=4))
        epool = ctx.enter_context(tc.tile_pool(name="epool", bufs=3))
        opool = ctx.enter_context(tc.tile_pool(name="opool", bufs=2))
        rpool = ctx.enter_context(tc.tile_pool(name="rpool", bufs=2))

        psH = ctx.enter_context(tc.tile_pool(name="psH", bufs=2, space=bass.MemorySpace.PSUM))
        psA = ctx.enter_context(tc.tile_pool(name="psA", bufs=2, space=bass.MemorySpace.PSUM))
        psU = ctx.enter_context(tc.tile_pool(name="psU", bufs=1, space=bass.MemorySpace.PSUM))
        psZ = ctx.enter_context(tc.tile_pool(name="psZ", bufs=1, space=bass.MemorySpace.PSUM))

        # DMA triggers cost ~650ns on every engine's queue; split them across
        # SP (consts, outputs) and Pool (bulk x) so they issue concurrently.
        dma = nc.sync.dma_start
        dmax = nc.gpsimd.dma_start

        # constants + first superstep, ordered so the first GEMM matmul
        # (needs w13[dc2=0] + xt0 first half) unblocks as early as possible
        w13_sb = consts.tile([128, 2, 2, 2 * H], dt.float8e4)
        dma(out=w13_sb[:, 0], in_=w13_d[:, 0])
        # PE p-state warmup: ~3.4us of dummy matmuls on zeroed tiles while the
        # first DMAs land, so the real GEMM starts at 2.4GHz instead of 0.65
        wz_sb = consts.tile([128, 2, 128], dt.float8e4)
        nc.gpsimd.memset(wz_sb[:], 0.0)
        xz_sb = consts.tile([128, 2, 512], dt.float8e4)
        nc.gpsimd.memset(xz_sb[:], 0.0)
        hz = psH.tile([128, 2, 512], dt.float32, tag="h", name="hz")
        for _ in range(16):
            nc.tensor.matmul(hz[:, 0, :], wz_sb[:], xz_sb[:],
                             start=True, stop=True, perf_mode=DR)

        xt_tiles = {}
        xa_tiles = {}

        def fetch(ss, split=False):
            if ss >= SS or ss in xt_tiles:
                return
            xtt = xtp.tile([128, DC, NI], dt.float8e4, tag="xt", name="xtt")
            if split:
                dmax(out=xtt[:, 0:2, :], in_=xt_d[ss, :, 0:2, :])
                dmax(out=xtt[:, 2:4, :], in_=xt_d[ss, :, 2:4, :])
            else:
                dmax(out=xtt[:], in_=xt_d[ss])
            xt_tiles[ss] = xtt
            if fp8:
                xat = xap.tile([128, 4, 2, D], dt.float8e4, tag="xa", name="xat")
            else:
                xat = xap.tile([128, 8, D], dt.bfloat16, tag="xa", name="xat")
            dmax(out=xat[:], in_=xa_d[ss])
            xa_tiles[ss] = xat

        fetch(0, split=True)
        b13_sb = consts.tile([128, DC], dt.float32)
        dma(out=b13_sb[:], in_=b13_d[:])
        dma(out=w13_sb[:, 1], in_=w13_d[:, 1])
        w2_sb = consts.tile([128, 2, K], dt.bfloat16)
        dma(out=w2_sb[:], in_=w2_d[:])
        if fp8:
            ones_sb = consts.tile([128, 2, 1], dt.float8e4)
        else:
            ones_sb = consts.tile([128, 1], dt.bfloat16)
        dma(out=ones_sb[:], in_=ones_d[:])
        fetch(1)

        # state carried across supersteps
        apts = {}       # ss -> (apt0, apt1)  [128, 2, 512] bf16 (h-chunk 0, 1)
        a_ps_of = {}    # ss -> psum [128, 8*K] logits
        e_of = {}       # ss -> e tile (bf16) or (e1, e2) fp8-padded
        uz_of = {}      # bag -> (u_ps, z_ps)

        def a_w2_all(ss):
            """A@W2 for all 8 chunks of superstep ss (16 small matmuls that
            stream at the bf16 weight-load rate, ~29ns each)."""
            a_ps = a_ps_of.setdefault(
                ss, psA.tile([128, 8 * K], dt.float32, tag="a", name="a_ps"))
            apt0, apt1 = apts[ss]
            for c in range(8):
                half, cc = c // 4, c % 4
                nc.tensor.matmul(
                    a_ps[:, K * c:K * (c + 1)],
                    apt0[:, half, cc * 128:(cc + 1) * 128],
                    w2_sb[:, 0, :], start=True, stop=False,
                )
                nc.tensor.matmul(
                    a_ps[:, K * c:K * (c + 1)],
                    apt1[:, half, cc * 128:(cc + 1) * 128],
                    w2_sb[:, 1, :], start=False, stop=True,
                )

        def do_exp(ss):
            """exp of superstep ss's logits; alloc u/z at bag start."""
            a_ps = a_ps_of.pop(ss)
            del apts[ss]
            e_b = epool.tile([128, 8, K], dt.bfloat16, tag="e", name="e_b")
            nc.scalar.activation(e_b[:], a_ps[:], AF.Exp)
            if fp8:
                # walrus rejects DoubleRow lhsT narrower than 16 cols, so e1/e2
                # are zero-padded [*, 8, 16]; rows 4..15 of u/z stay zero.
                e1 = epool.tile([128, 8, 16], dt.float8e4, tag="e1", name="e1")
                nc.gpsimd.memset(e1[:, :, K:], 0.0)
                nc.vector.tensor_scalar_add(out=e1[:, :, 0:K], in0=e_b[:], scalar1=0.0)
                e2 = epool.tile([128, 8, 16], dt.float8e4, tag="e2", name="e2")
                nc.gpsimd.memset(e2[:, :, K:], 0.0)
                nc.vector.scalar_tensor_tensor(
                    out=e2[:, :, 0:K], in0=e1[:, :, 0:K], scalar=-1.0, in1=e_b[:],
                    op0=mybir.AluOpType.mult, op1=mybir.AluOpType.add,
                )
                e_of[ss] = (e1, e2)
            else:
                e_of[ss] = e_b
            if ss % 2 == 0:
                if fp8:
                    u_ps = psU.tile([16, D], dt.float32, tag="u", name="u_ps")
                    z_ps = psZ.tile([16, 1], dt.float32, tag="z", name="z_ps")
                else:
                    u_ps = psU.tile([K, D], dt.float32, tag="u", name="u_ps")
                    z_ps = psZ.tile([K, 1], dt.float32, tag="z", name="z_ps")
                uz_of[ss // 2] = (u_ps, z_ps)

        def wsum_half(ss, h):
            """Half h of superstep ss's exp-weighted sum + normalizer.
            The u/z accumulation group spans the bag (both supersteps)."""
            u_ps, z_ps = uz_of[ss // 2]
            xat = xa_tiles[ss]
            first = ss % 2 == 0 and h == 0
            last = ss % 2 == 1 and h == 1
            if fp8:
                e1, e2 = e_of[ss]
                for pr in (2 * h, 2 * h + 1):  # chunk-pairs 0..3
                    firstp, lastp = first and pr == 2 * h, last and pr == 2 * h + 1
                    nc.tensor.matmul(u_ps[:], e1[:, 2 * pr:2 * pr + 2, :], xat[:, pr],
                                     start=firstp, stop=False, perf_mode=DR)
                    nc.tensor.matmul(u_ps[:], e2[:, 2 * pr:2 * pr + 2, :], xat[:, pr],
                                     start=False, stop=lastp, perf_mode=DR)
                    nc.tensor.matmul(z_ps[:], e1[:, 2 * pr:2 * pr + 2, :], ones_sb[:],
                                     start=firstp, stop=False, perf_mode=DR)
                    nc.tensor.matmul(z_ps[:], e2[:, 2 * pr:2 * pr + 2, :], ones_sb[:],
                                     start=False, stop=lastp, perf_mode=DR)
            else:
                e_b = e_of[ss]
                for c in range(4 * h, 4 * h + 4):  # chunks 0..7
                    firstc, lastc = first and c == 0, last and c == 7
                    nc.tensor.matmul(u_ps[:], e_b[:, c, :], xat[:, c],
                                     start=firstc, stop=lastc)
                    nc.tensor.matmul(z_ps[:], e_b[:, c, :], ones_sb[:],
                                     start=firstc, stop=lastc)
            if last:
                del e_of[ss - 1], e_of[ss]
                del xa_tiles[ss - 1], xa_tiles[ss]

        def finish(bag):
            u_ps, z_ps = uz_of.pop(bag)
            r_sb = rpool.tile([K, 1], dt.float32, tag="r", name="r_sb")
            nc.vector.reciprocal(out=r_sb[:], in_=z_ps[0:K, :])
            o_sb = opool.tile([K, D], dt.float32, tag="o", name="o_sb")
            nc.vector.tensor_scalar_mul(out=o_sb[:], in0=u_ps[0:K, :], scalar1=r_sb[:])
            dma(out=out_d[bag], in_=o_sb[:])

        for ss in range(SS):
            fetch(ss + 2)
            xtt = xt_tiles.pop(ss)
            tanh_of = {}
            lastss = ss == SS - 1
            for k, bp in enumerate((0, 2, 1, 3)):
                h2 = psH.tile([128, 2, 512], dt.float32, tag="h", name="h2")
                for half in range(2):
                    for dc2 in range(2):
                        nc.tensor.matmul(
                            h2[:, half, :],
                            w13_sb[:, dc2, :, bp * 128:(bp + 1) * 128],
                            xtt[:, 2 * dc2:2 * dc2 + 2, half * 512:(half + 1) * 512],
                            start=(dc2 == 0), stop=(dc2 == 1),
                            perf_mode=DR,
                        )
                # deferred PE work rides between the GEMM's matmul groups;
                # exp + e-splits are queued at superstep start so the weighted
                # sum two supersteps later never waits on them
                if k == 0 and ss >= 1:
                    a_w2_all(ss - 1)
                    do_exp(ss - 1)
                elif k in (1, 2) and ss >= 2:
                    wsum_half(ss - 2, k - 1)
                elif k == 3 and lastss:
                    wsum_half(ss - 1, 0)
                ts = tsp.tile([128, 2, 512], dt.bfloat16, tag="ts", name="ts")
                nc.scalar.activation(
                    ts[:], h2[:], AF.Tanh, bias=b13_sb[:, bp:bp + 1],
                    scale=(1.0 / 16.0 if bp < 2 else 0.5 / 16.0),
                )
                tanh_of[bp] = ts
                if k == 1:  # t(0) and s(2) ready -> gate h-chunk 0
                    apt0 = app.tile([128, 2, 512], dt.bfloat16, tag="ap", name="apt0")
                    nc.vector.scalar_tensor_tensor(
                        out=apt0[:], in0=tanh_of[2][:], scalar=1.0, in1=tanh_of[0][:],
                        op0=mybir.AluOpType.add, op1=mybir.AluOpType.mult,
                    )
                if k == 3:
                    apt1 = app.tile([128, 2, 512], dt.bfloat16, tag="ap", name="apt1")
                    nc.vector.scalar_tensor_tensor(
                        out=apt1[:], in0=tanh_of[3][:], scalar=1.0, in1=tanh_of[1][:],
                        op0=mybir.AluOpType.add, op1=mybir.AluOpType.mult,
                    )
                    apts[ss] = (apt0, apt1)
            if ss >= 3 and ss % 2 == 1:
                finish(ss // 2 - 1)

        # epilogue: drain the last superstep and bag
        wsum_half(SS - 2, 1)
        a_w2_all(SS - 1)
        do_exp(SS - 1)
        wsum_half(SS - 1, 0)
        wsum_half(SS - 1, 1)
        finish(BPC - 1)

    nc.compile()
    return nc


def get_nc():
    key = "nc_" + MODE
    if key not in _CACHE:
        _CACHE[key] = _build_nc(MODE)
    return _CACHE[key]


def make_in_maps(x, W1, b1, W3, b3, W2, b2):
    x = np.asarray(x, dtype=np.float32)
    W1 = np.asarray(W1, dtype=np.float32)
    W3 = np.asarray(W3, dtype=np.float32)
    W2 = np.asarray(W2, dtype=np.float32)
    b1 = np.asarray(b1, dtype=np.float32)
    b3 = np.asarray(b3, dtype=np.float32)
    fp8 = MODE == "fp8"

    w13 = np.concatenate([W1, W3], axis=1)          # [512, 512]
    # [p, dc2, r, h'] = 16*w13[dc2*256 + r*128 + p, h']
    w13_t = np.ascontiguousarray(
        (16.0 * w13).reshape(2, 2, 128, 2 * H).transpose(2, 0, 1, 3)
    ).astype(_FP8)
    w2_t = np.ascontiguousarray(
        (0.5 * W2).reshape(2, 128, K).transpose(1, 0, 2)
    ).astype(_BF16)
    b13 = np.concatenate([b1, 0.5 * b3]).reshape(DC, 128).T
    b13 = np.ascontiguousarray(b13, dtype=np.float32)
    if fp8:
        ones = np.ones((128, 2, 1), dtype=_FP8)
    else:
        ones = np.ones((128, 1), dtype=_BF16)

    in_maps = []
    for cid in range(NCORES):
        xc = x[cid * R:(cid + 1) * R]               # [8192, 512] fp32
        # xt[ss, p, 2*dc2+r, n] = x[ss*1024+n, dc2*256+r*128+p]
        xt_np = np.ascontiguousarray(
            xc.T.reshape(2, 2, 128, SS, NI).transpose(3, 2, 0, 1, 4).reshape(SS, 128, DC, NI)
        ).astype(_FP8)
        if fp8:
            xa_np = np.ascontiguousarray(
                xc.reshape(SS, 4, 2, 128, D).transpose(0, 3, 1, 2, 4)
            ).astype(_FP8)
        else:
            xa_np = np.ascontiguousarray(
                xc.reshape(SS, 8, 128, D).transpose(0, 2, 1, 3)
            ).astype(_BF16)
        in_maps.append(
            {"xt": xt_np, "xa": xa_np, "w13": w13_t, "w2": w2_t,
             "b13": b13, "ones": ones}
        )
    return in_maps


def kernel(x, W1, b1, W3, b3, W2, b2, bag_lengths):
    from concourse.bass_utils import run_bass_kernel_spmd

    nc = get_nc()
    in_maps = make_in_maps(x, W1, b1, W3, b3, W2, b2)
    res = run_bass_kernel_spmd(nc, in_maps, list(range(NCORES)))
    out = np.empty((B, K * D), dtype=np.float32)
    for c in range(NCORES):
        out[c * BPC:(c + 1) * BPC] = res.results[c]["out"].reshape(BPC, K * D)
    return out


# revision 25
# speedup vs baseline: 1.1239x; 1.0231x over previous
"""MIL gated-attention pooling kernel for Trainium2 (8 NeuronCores, SPMD).

Problem (per reference):
    A_pre = tanh(x@W1 + b1) * sigmoid(x@W3 + b3)      # [N, H]
    A     = A_pre @ W2 + b2                           # [N, K]
    P     = softmax over instances per (bag, head)    # [B, K, L]
    out   = einsum('bkl,bld->bkd', P, x) -> [B, K*D]

Shapes hardcoded: B=32 bags, L=2048 instances/bag, D=512, H=256, K=4.
Sharding: data-parallel over bags, 4 bags (8192 rows) per core, weights
replicated. No cross-core communication.

Device algorithm per core (v3):
  - 8 supersteps of 1024 instances; gated-MLP hidden tiles live in 2-bank
    PSUM tiles [128, 2, 512] so each tanh covers 1024 columns in ONE
    activation instruction (per-instruction ACT overhead is ~185ns).
  - sigmoid(h) = 0.5*(1+tanh(h/2)); 0.5 folded into W2 host-side; the
    gate (1+s)*t is ONE DVE scalar_tensor_tensor op.
  - b2 and softmax max-subtraction dropped (both cancel in softmax).
  - all DMA triggers issue from the idle Pool engine (~25ns each vs
    ~600ns on SP, whose serial trigger stream delayed the first matmul
    by ~8us in v2).
  - exp() runs per superstep so the weighted sum trails the GEMM by two
    supersteps and the epilogue is short.
  - A@W2 and weighted-sum matmuls are interleaved between the GEMM's
    DoubleRow matmuls to keep the PE stream dense (p-state: idle gaps
    drop the PE clock 2.4 -> 1.2GHz).
  - MODE "bf16": x for the weighted sum streams as bf16 [12.6MB/core].
    MODE "fp8": it streams as fp8 [8.4MB/core] and exp-weights are split
    e = e1 + e2 (both fp8, zero-padded to 16 lanes for walrus' DoubleRow
    codegen) so the weighted sum loses only x's fp8 quantization.
"""

import numpy as np
import ml_dtypes
from contextlib import ExitStack

B, L, D, H, K = 32, 2048, 512, 256, 4
NCORES = 8
BPC = B // NCORES       # bags per core = 4
R = BPC * L             # rows per core = 8192
SS = 8                  # supersteps per core (1024 instances each)
NI = 1024               # instances per superstep
DC = D // 128           # contraction chunks = 4
NCH = L // 128          # 128-row chunks per bag = 16

_BF16 = ml_dtypes.bfloat16
_FP8 = ml_dtypes.float8_e4m3

MODE = "fp8"            # "bf16" (safe) | "fp8" (fast; err ~1.74e-2 vs 2e-2 gate)
_CACHE = {}


def _build_nc(mode):
    import concourse.bacc as bacc
    import concourse.tile as tile
    import concourse.mybir as mybir
    import concourse.bass as bass

    dt = mybir.dt
    AF = mybir.ActivationFunctionType
    DR = mybir.MatmulPerfMode.DoubleRow
    fp8 = mode == "fp8"

    nc = bacc.Bacc("TRN2", target_bir_lowering=False, debug=False)

    # xt[ss, p, 2*dc2+r, n] = fp8(x[ss*1024+n, dc2*256+r*128+p])
    xt_d = nc.dram_tensor("xt", [SS, 128, DC, NI], dt.float8e4, kind="ExternalInput").ap()
    if fp8:
        # q1[ss, p, pr, j, d] = fp8(x[ss*1024+(2*pr+j)*128+p, d])
        xa_d = nc.dram_tensor("xa", [SS, 128, 4, 2, D], dt.float8e4, kind="ExternalInput").ap()
        ones_d = nc.dram_tensor("ones", [128, 2, 1], dt.float8e4, kind="ExternalInput").ap()
    else:
        # xa[ss, p, c, d] = bf16(x[ss*1024+c*128+p, d])
        xa_d = nc.dram_tensor("xa", [SS, 128, 8, D], dt.bfloat16, kind="ExternalInput").ap()
        ones_d = nc.dram_tensor("ones", [128, 1], dt.bfloat16, kind="ExternalInput").ap()
    # w13[p, dc2, r, h'] = 16*[W1|W3][dc2*256+r*128+p, h']
    w13_d = nc.dram_tensor("w13", [128, 2, 2, 2 * H], dt.float8e4, kind="ExternalInput").ap()
    w2_d = nc.dram_tensor("w2", [128, 2, K], dt.bfloat16, kind="ExternalInput").ap()
    b13_d = nc.dram_tensor("b13", [128, DC], dt.float32, kind="ExternalInput").ap()
    out_d = nc.dram_tensor("out", [BPC, K, D], dt.float32, kind="ExternalOutput").ap()

    with tile.TileContext(nc) as tc, ExitStack() as ctx:
        consts = ctx.enter_context(tc.tile_pool(name="consts", bufs=1))
        xtp = ctx.enter_context(tc.tile_pool(name="xtp", bufs=3))
        xap = ctx.enter_context(tc.tile_pool(name="xap", bufs=6))
        tsp = ctx.enter_context(tc.tile_pool(name="tsp", bufs=4))
        app = ctx.enter_context(tc.tile_pool(name="app", bufs=4))
        epool = ctx.enter_context(tc.tile_pool(name="epool", bufs=3))
        opool = ctx.enter_context(tc.tile_pool(name="opool", bufs=2))
        rpool = ctx.enter_context(tc.tile_pool(name="rpool", bufs=2))

        psH = ctx.enter_context(tc.tile_pool(name="psH", bufs=2, space=bass.MemorySpace.PSUM))
        psA = ctx.enter_context(tc.tile_pool(name="psA", bufs=2, space=bass.MemorySpace.PSUM))
        psU = ctx.enter_context(tc.tile_pool(name="psU", bufs=1, space=bass.MemorySpace.PSUM))
        psZ = ctx.enter_context(tc.tile_pool(name="psZ", bufs=1, space=bass.MemorySpace.PSUM))

        # DMA triggers cost ~650ns on every engine's queue; split them across
        # SP (consts, outputs) and Pool (bulk x) so they issue concurrently.
        dma = nc.sync.dma_start
        dmax = nc.gpsimd.dma_start

        # constants + first superstep, ordered so the first GEMM matmul
        # (needs w13[dc2=0] + xt0 first half) unblocks as early as possible
        w13_sb = consts.tile([128, 2, 2, 2 * H], dt.float8e4)
        dma(out=w13_sb[:, 0], in_=w13_d[:, 0])
        # PE p-state warmup: ~3.4us of dummy matmuls on zeroed tiles while the
        # first DMAs land, so the real GEMM starts at 2.4GHz instead of 0.65
        wz_sb = consts.tile([128, 2, 128], dt.float8e4)
        nc.gpsimd.memset(wz_sb[:], 0.0)
        xz_sb = consts.tile([128, 2, 512], dt.float8e4)
        nc.gpsimd.memset(xz_sb[:], 0.0)
        hz = psH.tile([128, 2, 512], dt.float32, tag="h", name="hz")
        for _ in range(16):
            nc.tensor.matmul(hz[:, 0, :], wz_sb[:], xz_sb[:],
                             start=True, stop=True, perf_mode=DR)
        # preload the tanh/exp activation table (~1.3us) off the critical path
        tz_sb = consts.tile([128, 1], dt.bfloat16)
        nc.scalar.activation(tz_sb[:], wz_sb[:, 0, 0:1], AF.Tanh)

        xt_tiles = {}
        xa_tiles = {}

        def fetch(ss, split=False):
            if ss >= SS or ss in xt_tiles:
                return
            xtt = xtp.tile([128, DC, NI], dt.float8e4, tag="xt", name="xtt")
            if split:
                dmax(out=xtt[:, 0:2, :], in_=xt_d[ss, :, 0:2, :])
                dmax(out=xtt[:, 2:4, :], in_=xt_d[ss, :, 2:4, :])
            else:
                dmax(out=xtt[:], in_=xt_d[ss])
            xt_tiles[ss] = xtt
            if fp8:
                xat = xap.tile([128, 4, 2, D], dt.float8e4, tag="xa", name="xat")
            else:
                xat = xap.tile([128, 8, D], dt.bfloat16, tag="xa", name="xat")
            dmax(out=xat[:], in_=xa_d[ss])
            xa_tiles[ss] = xat

        fetch(0, split=True)
        b13_sb = consts.tile([128, DC], dt.float32)
        dma(out=b13_sb[:], in_=b13_d[:])
        dma(out=w13_sb[:, 1], in_=w13_d[:, 1])
        w2_sb = consts.tile([128, 2, K], dt.bfloat16)
        dma(out=w2_sb[:], in_=w2_d[:])
        if fp8:
            ones_sb = consts.tile([128, 2, 1], dt.float8e4)
        else:
            ones_sb = consts.tile([128, 1], dt.bfloat16)
        dma(out=ones_sb[:], in_=ones_d[:])
        fetch(1)

        # state carried across supersteps
        apts = {}       # ss -> (apt0, apt1)  [128, 2, 512] bf16 (h-chunk 0, 1)
        a_ps_of = {}    # ss -> psum [128, 8*K] logits
        e_of = {}       # ss -> e tile (bf16) or (e1, e2) fp8-padded
        e_b_of = {}     # ss -> bf16 exp tile (for half-granular epilogue)
        uz_of = {}      # bag -> (u_ps, z_ps)

        def a_w2_all(ss):
            """A@W2 for all 8 chunks of superstep ss (16 small matmuls that
            stream at the bf16 weight-load rate, ~29ns each)."""
            a_ps = a_ps_of.setdefault(
                ss, psA.tile([128, 8 * K], dt.float32, tag="a", name="a_ps"))
            apt0, apt1 = apts[ss]
            for c in range(8):
                half, cc = c // 4, c % 4
                nc.tensor.matmul(
                    a_ps[:, K * c:K * (c + 1)],
                    apt0[:, half, cc * 128:(cc + 1) * 128],
                    w2_sb[:, 0, :], start=True, stop=False,
                )
                nc.tensor.matmul(
                    a_ps[:, K * c:K * (c + 1)],
                    apt1[:, half, cc * 128:(cc + 1) * 128],
                    w2_sb[:, 1, :], start=False, stop=True,
                )

        def exp_half(ss, h, a_ps):
            """exp + e-split of chunk-half h of superstep ss's logits."""
            c0 = 4 * h
            if h == 0:
                e_b = epool.tile([128, 8, K], dt.bfloat16, tag="e", name="e_b")
                if fp8:
                    # walrus rejects DoubleRow lhsT narrower than 16 cols, so
                    # e1/e2 are zero-padded [*, 8, 16]; u/z rows 4..15 stay 0.
                    e1 = epool.tile([128, 8, 16], dt.float8e4, tag="e1", name="e1")
                    nc.gpsimd.memset(e1[:, :, K:], 0.0)
                    e2 = epool.tile([128, 8, 16], dt.float8e4, tag="e2", name="e2")
                    nc.gpsimd.memset(e2[:, :, K:], 0.0)
                    e_of[ss] = (e1, e2)
                    e_b_of[ss] = e_b
                else:
                    e_of[ss] = e_b
                    e_b_of[ss] = e_b
            e_b = e_b_of[ss]
            nc.scalar.activation(e_b[:, c0:c0 + 4, :], a_ps[:, K * c0:K * (c0 + 4)], AF.Exp)
            if fp8:
                e1, e2 = e_of[ss]
                nc.vector.tensor_scalar_add(
                    out=e1[:, c0:c0 + 4, 0:K], in0=e_b[:, c0:c0 + 4, :], scalar1=0.0)
                nc.vector.scalar_tensor_tensor(
                    out=e2[:, c0:c0 + 4, 0:K], in0=e1[:, c0:c0 + 4, 0:K], scalar=-1.0,
                    in1=e_b[:, c0:c0 + 4, :],
                    op0=mybir.AluOpType.mult, op1=mybir.AluOpType.add,
                )

        def do_exp(ss):
            """exp of superstep ss's logits; alloc u/z at bag start."""
            a_ps = a_ps_of.pop(ss)
            del apts[ss]
            e_b = epool.tile([128, 8, K], dt.bfloat16, tag="e", name="e_b")
            nc.scalar.activation(e_b[:], a_ps[:], AF.Exp)
            if fp8:
                e1 = epool.tile([128, 8, 16], dt.float8e4, tag="e1", name="e1")
                nc.gpsimd.memset(e1[:, :, K:], 0.0)
                nc.vector.tensor_scalar_add(out=e1[:, :, 0:K], in0=e_b[:], scalar1=0.0)
                e2 = epool.tile([128, 8, 16], dt.float8e4, tag="e2", name="e2")
                nc.gpsimd.memset(e2[:, :, K:], 0.0)
                nc.vector.scalar_tensor_tensor(
                    out=e2[:, :, 0:K], in0=e1[:, :, 0:K], scalar=-1.0, in1=e_b[:],
                    op0=mybir.AluOpType.mult, op1=mybir.AluOpType.add,
                )
                e_of[ss] = (e1, e2)
            else:
                e_of[ss] = e_b
            if ss % 2 == 0:
                alloc_uz(ss // 2)

        def alloc_uz(bag):
            if fp8:
                u_ps = psU.tile([16, D], dt.float32, tag="u", name="u_ps")
                z_ps = psZ.tile([16, 1], dt.float32, tag="z", name="z_ps")
            else:
                u_ps = psU.tile([K, D], dt.float32, tag="u", name="u_ps")
                z_ps = psZ.tile([K, 1], dt.float32, tag="z", name="z_ps")
            uz_of[bag] = (u_ps, z_ps)

        def wsum_half(ss, h):
            """Half h of superstep ss's exp-weighted sum + normalizer.
            The u/z accumulation group spans the bag (both supersteps)."""
            u_ps, z_ps = uz_of[ss // 2]
            xat = xa_tiles[ss]
            first = ss % 2 == 0 and h == 0
            last = ss % 2 == 1 and h == 1
            if fp8:
                e1, e2 = e_of[ss]
                for pr in (2 * h, 2 * h + 1):  # chunk-pairs 0..3
                    firstp, lastp = first and pr == 2 * h, last and pr == 2 * h + 1
                    nc.tensor.matmul(u_ps[:], e1[:, 2 * pr:2 * pr + 2, :], xat[:, pr],
                                     start=firstp, stop=False, perf_mode=DR)
                    nc.tensor.matmul(u_ps[:], e2[:, 2 * pr:2 * pr + 2, :], xat[:, pr],
                                     start=False, stop=lastp, perf_mode=DR)
                    nc.tensor.matmul(z_ps[:], e1[:, 2 * pr:2 * pr + 2, :], ones_sb[:],
                                     start=firstp, stop=False, perf_mode=DR)
                    nc.tensor.matmul(z_ps[:], e2[:, 2 * pr:2 * pr + 2, :], ones_sb[:],
                                     start=False, stop=lastp, perf_mode=DR)
            else:
                e_b = e_of[ss]
                for c in range(4 * h, 4 * h + 4):  # chunks 0..7
                    firstc, lastc = first and c == 0, last and c == 7
                    nc.tensor.matmul(u_ps[:], e_b[:, c, :], xat[:, c],
                                     start=firstc, stop=lastc)
                    nc.tensor.matmul(z_ps[:], e_b[:, c, :], ones_sb[:],
                                     start=firstc, stop=lastc)
            if last:
                del e_of[ss - 1], e_of[ss]
                e_b_of.pop(ss - 1, None), e_b_of.pop(ss, None)
                del xa_tiles[ss - 1], xa_tiles[ss]

        def finish(bag):
            u_ps, z_ps = uz_of.pop(bag)
            r_sb = rpool.tile([K, 1], dt.float32, tag="r", name="r_sb")
            nc.vector.reciprocal(out=r_sb[:], in_=z_ps[0:K, :])
            o_sb = opool.tile([K, D], dt.float32, tag="o", name="o_sb")
            nc.vector.tensor_scalar_mul(out=o_sb[:], in0=u_ps[0:K, :], scalar1=r_sb[:])
            dma(out=out_d[bag], in_=o_sb[:])

        for ss in range(SS):
            fetch(ss + 2)
            xtt = xt_tiles.pop(ss)
            tanh_of = {}
            lastss = ss == SS - 1
            for k, bp in enumerate((0, 2, 1, 3)):
                h2 = psH.tile([128, 2, 512], dt.float32, tag="h", name="h2")
                for half in range(2):
                    for dc2 in range(2):
                        nc.tensor.matmul(
                            h2[:, half, :],
                            w13_sb[:, dc2, :, bp * 128:(bp + 1) * 128],
                            xtt[:, 2 * dc2:2 * dc2 + 2, half * 512:(half + 1) * 512],
                            start=(dc2 == 0), stop=(dc2 == 1),
                            perf_mode=DR,
                        )
                # deferred PE work rides between the GEMM's matmul groups;
                # exp + e-splits are queued at superstep start so the weighted
                # sum two supersteps later never waits on them
                if k == 0 and ss >= 1:
                    a_w2_all(ss - 1)
                    do_exp(ss - 1)
                elif k in (1, 2) and ss >= 2:
                    wsum_half(ss - 2, k - 1)
                elif k == 3 and lastss:
                    wsum_half(ss - 1, 0)
                ts = tsp.tile([128, 2, 512], dt.bfloat16, tag="ts", name="ts")
                nc.scalar.activation(
                    ts[:], h2[:], AF.Tanh, bias=b13_sb[:, bp:bp + 1],
                    scale=(1.0 / 16.0 if bp < 2 else 0.5 / 16.0),
                )
                tanh_of[bp] = ts
                if k == 1:  # t(0) and s(2) ready -> gate h-chunk 0
                    apt0 = app.tile([128, 2, 512], dt.bfloat16, tag="ap", name="apt0")
                    nc.vector.scalar_tensor_tensor(
                        out=apt0[:], in0=tanh_of[2][:], scalar=1.0, in1=tanh_of[0][:],
                        op0=mybir.AluOpType.add, op1=mybir.AluOpType.mult,
                    )
                if k == 3:
                    apt1 = app.tile([128, 2, 512], dt.bfloat16, tag="ap", name="apt1")
                    nc.vector.scalar_tensor_tensor(
                        out=apt1[:], in0=tanh_of[3][:], scalar=1.0, in1=tanh_of[1][:],
                        op0=mybir.AluOpType.add, op1=mybir.AluOpType.mult,
                    )
                    apts[ss] = (apt0, apt1)
            if ss >= 3 and ss % 2 == 1:
                finish(ss // 2 - 1)

        # epilogue: drain the last superstep (per-half A@W2 groups stay
        # closed — the PSUM bank supports only one open accumulation group —
        # and exp/e-split halves interleave with the remaining PE work)
        ls = SS - 1
        a_ps7 = psA.tile([128, 8 * K], dt.float32, tag="a", name="a_ps")
        apt0, apt1 = apts.pop(ls)
        def a_w2_chunk(c):
            half, cc = c // 4, c % 4
            nc.tensor.matmul(a_ps7[:, K * c:K * (c + 1)],
                             apt0[:, half, cc * 128:(cc + 1) * 128],
                             w2_sb[:, 0, :], start=True, stop=False)
            nc.tensor.matmul(a_ps7[:, K * c:K * (c + 1)],
                             apt1[:, half, cc * 128:(cc + 1) * 128],
                             w2_sb[:, 1, :], start=False, stop=True)
        for c in range(4):
            a_w2_chunk(c)
        exp_half(ls, 0, a_ps7)
        for c in range(4, 8):
            a_w2_chunk(c)
        wsum_half(SS - 2, 1)
        exp_half(ls, 1, a_ps7)
        wsum_half(ls, 0)
        wsum_half(ls, 1)
        finish(BPC - 1)

    nc.compile()
    return nc


def get_nc():
    key = "nc_" + MODE
    if key not in _CACHE:
        _CACHE[key] = _build_nc(MODE)
    return _CACHE[key]


def make_in_maps(x, W1, b1, W3, b3, W2, b2):
    x = np.asarray(x, dtype=np.float32)
    W1 = np.asarray(W1, dtype=np.float32)
    W3 = np.asarray(W3, dtype=np.float32)
    W2 = np.asarray(W2, dtype=np.float32)
    b1 = np.asarray(b1, dtype=np.float32)
    b3 = np.asarray(b3, dtype=np.float32)
    fp8 = MODE == "fp8"

    w13 = np.concatenate([W1, W3], axis=1)          # [512, 512]
    # [p, dc2, r, h'] = 16*w13[dc2*256 + r*128 + p, h']
    w13_t = np.ascontiguousarray(
        (16.0 * w13).reshape(2, 2, 128, 2 * H).transpose(2, 0, 1, 3)
    ).astype(_FP8)
    w2_t = np.ascontiguousarray(
        (0.5 * W2).reshape(2, 128, K).transpose(1, 0, 2)
    ).astype(_BF16)
    b13 = np.concatenate([b1, 0.5 * b3]).reshape(DC, 128).T
    b13 = np.ascontiguousarray(b13, dtype=np.float32)
    if fp8:
        ones = np.ones((128, 2, 1), dtype=_FP8)
    else:
        ones = np.ones((128, 1), dtype=_BF16)

    in_maps = []
    for cid in range(NCORES):
        xc = x[cid * R:(cid + 1) * R]               # [8192, 512] fp32
        # xt[ss, p, 2*dc2+r, n] = x[ss*1024+n, dc2*256+r*128+p]
        xt_np = np.ascontiguousarray(
            xc.T.reshape(2, 2, 128, SS, NI).transpose(3, 2, 0, 1, 4).reshape(SS, 128, DC, NI)
        ).astype(_FP8)
        if fp8:
            xa_np = np.ascontiguousarray(
                xc.reshape(SS, 4, 2, 128, D).transpose(0, 3, 1, 2, 4)
            ).astype(_FP8)
        else:
            xa_np = np.ascontiguousarray(
                xc.reshape(SS, 8, 128, D).transpose(0, 2, 1, 3)
            ).astype(_BF16)
        in_maps.append(
            {"xt": xt_np, "xa": xa_np, "w13": w13_t, "w2": w2_t,
             "b13": b13, "ones": ones}
        )
    return in_maps


def kernel(x, W1, b1, W3, b3, W2, b2, bag_lengths):
    from concourse.bass_utils import run_bass_kernel_spmd

    nc = get_nc()
    in_maps = make_in_maps(x, W1, b1, W3, b3, W2, b2)
    res = run_bass_kernel_spmd(nc, in_maps, list(range(NCORES)))
    out = np.empty((B, K * D), dtype=np.float32)
    for c in range(NCORES):
        out[c * BPC:(c + 1) * BPC] = res.results[c]["out"].reshape(BPC, K * D)
    return out
